# revision 18
# baseline (speedup 1.0000x reference)
"""Trainium2 Bass kernel for nn_ContinuousAttention (B=32, L=2999, D=512, NB=16).

Math (per example b):
    u      = W_enc @ q[b]                      (D,)
    s[l]   = keys[b,l,:] . u / sqrt(D)         (L,)   raw scores
    w[l]   = exp(s[l])                          -- no max-subtraction needed:
                                                  s ~ N(0,1), |s| < ~6, exp safe
    Z      = sum w;  S1 = sum w*pos;  S2 = sum w*pos^2
    mu     = S1/Z;  var = clip(S2/Z - mu^2, 1e-7)
    tv_j   = var + basis_sigma_j^2
    r_j    = (1/sqrt(2pi)) / sqrt(tv_j) * exp(-0.5 (mu - mu_j)^2 / tv_j)
    BmatT  = G^T @ values[b]                   (NB, D)  [= (values^T G)^T]
    c[b]   = r . BmatT                         (D,)

Sharding: data-parallel over batch, 4 examples per core x 8 cores.
The only large traffic is keys+values (24.6 MB each per core), streamed
once from HBM in 1 MiB chunks at the per-core HBM roofline (~358 GB/s,
so ~140 us/core is the floor). Score dot-products are spread over three
engines (GpSimd mul + DVE reduce on even sub-tiles, DVE mul + ACT
accumulate-reduce on odd ones); Bmat runs on the PE with G stationary;
the softmax-statistics partition reduction is a ones-vector PE matmul
done per example so it overlaps the remaining stream. Only a short
scalar chain (~20 tiny ops) plus the final combines remain after the
last DMA.
"""

import numpy as np
from contextlib import ExitStack

import concourse.bass as bass
import concourse.bacc as bacc
import concourse.tile as tile
from concourse import mybir
from concourse.bass_utils import run_bass_kernel_spmd

F32 = mybir.dt.float32
AF = mybir.ActivationFunctionType
ALU = mybir.AluOpType

B, L, D, NB = 32, 2999, 512, 16
NCORES = 8
PER = B // NCORES              # 4 examples per core
NT = (L + 127) // 128          # 24 row-tiles of 128
LAST = L - (NT - 1) * 128      # 55 rows in the last tile
INV_SQRT_D = float(1.0 / np.sqrt(float(D)))
INV_SQRT_2PI = float(1.0 / np.sqrt(2.0 * np.pi))
NEG_BIG = -1.0e4               # pad score; exp(NEG_BIG/sqrt(D)) == 0 in f32

# 1 MiB DMA chunks: (row0, n_full_128_subtiles, tail_rows)
CHUNKS = []
_r = 0
while _r < L:
    _full = min(4, (L - _r) // 128)
    _tail = 0 if _full == 4 else L - _r - _full * 128
    CHUNKS.append((_r, _full, _tail))
    _r += _full * 128 + _tail


def _build_bass():
    # Bacc (not raw Bass): its compile pipeline splits multi-wait sync infos
    # into event semaphores, which the TRN2 BIR verifier requires for the
    # Tile kernel-tail drain.
    nc = bacc.Bacc(None, target_bir_lowering=False)
    keys_t = nc.declare_dram_parameter("keys", [PER, L, D], F32, isOutput=False)
    vals_t = nc.declare_dram_parameter("values", [PER, L, D], F32, isOutput=False)
    q_t = nc.declare_dram_parameter("q", [PER, D], F32, isOutput=False)
    W_t = nc.declare_dram_parameter("W", [D, D], F32, isOutput=False)
    G_t = nc.declare_dram_parameter("G", [L, NB], F32, isOutput=False)
    bmu_t = nc.declare_dram_parameter("bmu", [PER, NB], F32, isOutput=False)
    bsig_t = nc.declare_dram_parameter("bsig", [PER, NB], F32, isOutput=False)
    id_t = nc.declare_dram_parameter("ident", [128, 128], F32, isOutput=False)
    pos_t = nc.declare_dram_parameter("post", [128, NT], F32, isOutput=False)
    out_t = nc.declare_dram_parameter("out", [PER, D], F32, isOutput=True)

    with ExitStack() as ctx:
        tc = ctx.enter_context(tile.TileContext(nc))
        const = ctx.enter_context(tc.tile_pool(name="const", bufs=1))
        kpool = ctx.enter_context(tc.tile_pool(name="kpool", bufs=8))
        vpool = ctx.enter_context(tc.tile_pool(name="vpool", bufs=8))
        spool = ctx.enter_context(tc.tile_pool(name="spool", bufs=4))
        pwork = ctx.enter_context(tc.tile_pool(name="pwork", bufs=3, space="PSUM"))
        pbm = ctx.enter_context(tc.tile_pool(name="pbm", bufs=4, space="PSUM"))

        # ------- stream prefetch: first two chunk-pairs of example 0 issue on
        # the ACT HWDGE ring before anything else, so both DMA rings start
        # moving bytes at t~0 instead of serializing behind one queue.
        pre_k, pre_v = {}, {}
        for ci in (0, 1):
            r0, nfull, _ = CHUNKS[ci]
            kt = kpool.tile([128, 4, D], F32, tag="ktile", name=f"pre_k{ci}")
            vt = vpool.tile([128, 4, D], F32, tag="vtile", name=f"pre_v{ci}")
            nc.scalar.dma_start(
                out=kt[:, 0:nfull, :],
                in_=keys_t[0, r0 : r0 + nfull * 128, :].rearrange(
                    "(t p) d -> p t d", p=128
                ),
            )
            nc.scalar.dma_start(
                out=vt[:, 0:nfull, :],
                in_=vals_t[0, r0 : r0 + nfull * 128, :].rearrange(
                    "(t p) d -> p t d", p=128
                ),
            )
            pre_k[ci], pre_v[ci] = kt, vt

        # ------- constants (ACT HWDGE ring; the sync ring is for streams) ----
        I_sb = const.tile([128, 128], F32, tag="I")
        nc.scalar.dma_start(out=I_sb, in_=id_t[:, :])
        pos_sb = const.tile([128, NT], F32, tag="pos")
        nc.scalar.dma_start(out=pos_sb, in_=pos_t[:, :])
        ones_sb = const.tile([128, 1], F32, tag="ones")
        nc.vector.memset(ones_sb, 1.0)
        bmu_sb = const.tile([PER, NB], F32, tag="bmu")
        nc.scalar.dma_start(out=bmu_sb, in_=bmu_t[:, :])
        bsig_sb = const.tile([PER, NB], F32, tag="bsig")
        nc.scalar.dma_start(out=bsig_sb, in_=bsig_t[:, :])
        sig2_sb = const.tile([PER, NB], F32, tag="sig2")
        nc.vector.tensor_mul(sig2_sb, bsig_sb, bsig_sb)

        # G (L, NB) -> (128, NT, NB); last tile has LAST valid rows
        G_sb = const.tile([128, NT, NB], F32, tag="G")
        nc.scalar.dma_start(
            out=G_sb[:, 0 : NT - 1, :],
            in_=G_t[0 : (NT - 1) * 128, :].rearrange("(t p) n -> p t n", p=128),
        )
        nc.scalar.dma_start(out=G_sb[:LAST, NT - 1, :], in_=G_t[(NT - 1) * 128 : L, :])

        q_sb = const.tile([PER, D], F32, tag="q")
        nc.scalar.dma_start(out=q_sb, in_=q_t[:, :])

        # W (D, D) row-tiled: w_sb[p, dt, e] = W[dt*128+p, e]
        w_sb = const.tile([128, 4, D], F32, tag="W")
        nc.scalar.dma_start(out=w_sb, in_=W_t[:, :].rearrange("(t p) e -> p t e", p=128))

        # qT[p, et, b] = q[b, et*128+p]
        qT_sb = const.tile([128, 4, PER], F32, tag="qT")
        for et in range(4):
            tp = pwork.tile([128, PER], F32, tag="pwork")
            nc.tensor.transpose(tp, q_sb[:, et * 128 : (et + 1) * 128], I_sb[:PER, :PER])
            nc.vector.tensor_copy(out=qT_sb[:, et, :], in_=tp)

        # wT_sb[p, et, d] = W[d, et*128+p]  (transpose of W, e on partitions)
        wT_sb = const.tile([128, 4, D], F32, tag="WT")
        for dt in range(4):
            for et in range(4):
                tp = pwork.tile([128, 128], F32, tag="pwork")
                nc.tensor.transpose(tp, w_sb[:, dt, et * 128 : (et + 1) * 128], I_sb)
                nc.vector.tensor_copy(
                    out=wT_sb[:, et, dt * 128 : (dt + 1) * 128], in_=tp
                )

        # U[p, dm, b] = u_b[dm*128+p] = sum_e W[dm*128+p, e] q[b, e]
        U_sb = const.tile([128, 4, PER], F32, tag="U")
        for dm in range(4):
            up = pwork.tile([128, PER], F32, tag="pwork")
            for et in range(4):
                nc.tensor.matmul(
                    up,
                    lhsT=wT_sb[:, et, dm * 128 : (dm + 1) * 128],
                    rhs=qT_sb[:, et, :],
                    start=(et == 0),
                    stop=(et == 3),
                )
            nc.vector.tensor_copy(out=U_sb[:, dm, :], in_=up)

        # UT[b, d] = u_b[d]
        UT_sb = const.tile([PER, D], F32, tag="UT")
        for dm in range(4):
            tp = pwork.tile([PER, 128], F32, tag="pwork")
            nc.tensor.transpose(tp, U_sb[:, dm, :], I_sb)
            nc.vector.tensor_copy(out=UT_sb[:, dm * 128 : (dm + 1) * 128], in_=tp)

        # u_sb[p, b, d] = u_b[d] broadcast across all 128 partitions.
        # Step 1: select row b of UT into partition 0 (lhsT = I4 column b).
        # Step 2: outer-product with a ones row to replicate across partitions.
        ones_row = const.tile([1, 128], F32, tag="ones_row")
        nc.vector.memset(ones_row, 1.0)
        u_sb = const.tile([128, PER, D], F32, tag="u")
        for b in range(PER):
            ur_ps = pwork.tile([1, D], F32, tag="pwork", name=f"ur_ps{b}")
            nc.tensor.matmul(
                ur_ps, lhsT=I_sb[:PER, b : b + 1], rhs=UT_sb, start=True, stop=True
            )
            ur_sb = const.tile([1, D], F32, tag="ur", name=f"ur{b}")
            nc.vector.tensor_copy(out=ur_sb, in_=ur_ps)
            ub = pwork.tile([128, D], F32, tag="pwork", name=f"ub{b}")
            nc.tensor.matmul(ub, lhsT=ones_row, rhs=ur_sb, start=True, stop=True)
            nc.vector.tensor_copy(out=u_sb[:, b, :], in_=ub)

        # ---------------- main streams ----------------
        scores_sb = const.tile([128, PER, NT], F32, tag="scores")
        nc.vector.memset(scores_sb, NEG_BIG)
        # wst[p, b, s, t]: s=0 -> w, s=1 -> w*pos, s=2 -> w*pos^2
        # (b-major: each example's 3*NT stat block is contiguous)
        wst_sb = const.tile([128, PER, 3, NT], F32, tag="wst")
        # per-example partition-reduced stats land in st_all[0, :, b]
        st_all = const.tile([1, 3, PER], F32, tag="st_all")
        bm_ps = [
            pbm.tile([NB, D], F32, tag="pbm", name=f"bm_ps{b}") for b in range(PER)
        ]

        for b in range(PER):
            for ci, (r0, nfull, tailr) in enumerate(CHUNKS):
                if b == 0 and ci in pre_k:
                    k_tile, v_tile = pre_k[ci], pre_v[ci]
                else:
                    k_tile = kpool.tile([128, 4, D], F32, tag="ktile")
                    v_tile = vpool.tile([128, 4, D], F32, tag="vtile")
                    nc.sync.dma_start(
                        out=k_tile[:, 0:nfull, :],
                        in_=keys_t[b, r0 : r0 + nfull * 128, :].rearrange(
                            "(t p) d -> p t d", p=128
                        ),
                    )
                    nc.sync.dma_start(
                        out=v_tile[:, 0:nfull, :],
                        in_=vals_t[b, r0 : r0 + nfull * 128, :].rearrange(
                            "(t p) d -> p t d", p=128
                        ),
                    )
                if tailr:
                    nc.sync.dma_start(
                        out=k_tile[:tailr, nfull, :],
                        in_=keys_t[b, r0 + nfull * 128 : L, :],
                    )
                    nc.sync.dma_start(
                        out=v_tile[:tailr, nfull, :],
                        in_=vals_t[b, r0 + nfull * 128 : L, :],
                    )
                nsub = nfull + (1 if tailr else 0)
                for s in range(nsub):
                    t = r0 // 128 + s
                    P = 128 if s < nfull else tailr
                    # keys[l,:] . u  -> score per row, spread over three
                    # engines so each stays under the DMA pace: even
                    # sub-tiles GpSimd-mul + DVE-reduce, odd sub-tiles
                    # DVE-mul + ACT accumulate-reduce.
                    scr = spool.tile([128, D], F32, tag="scr")
                    if s % 2 == 0:
                        nc.gpsimd.tensor_mul(
                            scr[:P, :], k_tile[:P, s, :], u_sb[:P, b, :]
                        )
                        nc.vector.tensor_reduce(
                            out=scores_sb[:P, b, t : t + 1],
                            in_=scr[:P, :],
                            axis=mybir.AxisListType.X,
                            op=ALU.add,
                        )
                    else:
                        nc.vector.tensor_mul(
                            scr[:P, :], k_tile[:P, s, :], u_sb[:P, b, :]
                        )
                        nc.scalar.activation(
                            out=scr[:P, :],
                            in_=scr[:P, :],
                            func=AF.Copy,
                            accum_out=scores_sb[:P, b, t : t + 1],
                        )
                    nc.tensor.matmul(
                        bm_ps[b],
                        lhsT=G_sb[:P, t, :],
                        rhs=v_tile[:P, s, :],
                        start=(t == 0),
                        stop=(t == NT - 1),
                    )

            # per-example 128-wide stats work, overlapping later streams
            nc.scalar.activation(
                out=wst_sb[:, b, 0, :],
                in_=scores_sb[:, b, :],
                func=AF.Exp,
                scale=INV_SQRT_D,
            )
            nc.vector.tensor_mul(wst_sb[:, b, 1, :], wst_sb[:, b, 0, :], pos_sb)
            nc.vector.tensor_mul(wst_sb[:, b, 2, :], wst_sb[:, b, 1, :], pos_sb)
            st_ps = pwork.tile([1, 3, NT], F32, tag="pwork", name=f"st_ps{b}")
            nc.tensor.matmul(
                st_ps, lhsT=ones_sb, rhs=wst_sb[:, b, :, :], start=True, stop=True
            )
            nc.vector.tensor_reduce(
                out=st_all[:, :, b], in_=st_ps, axis=mybir.AxisListType.X, op=ALU.add
            )

        # ---------------- tail (short scalar chain) ----------------
        # (1, PER) stat rows -> (PER, 1) columns via tiny PE transposes
        zs = []
        for s in range(3):
            tp = pwork.tile([PER, 1], F32, tag="pwork", name=f"zt{s}")
            nc.tensor.matmul(
                tp, lhsT=st_all[:, s, :], rhs=I_sb[:1, :1], start=True, stop=True
            )
            z_sb = const.tile([PER, 1], F32, tag=f"zs{s}")
            nc.vector.tensor_copy(out=z_sb, in_=tp)
            zs.append(z_sb)
        Z_sb, S1_sb, S2_sb = zs

        rZ = const.tile([PER, 1], F32, tag="rZ")
        nc.vector.reciprocal(rZ, Z_sb)
        mu = const.tile([PER, 1], F32, tag="mu")
        nc.vector.tensor_mul(mu, S1_sb, rZ)
        e2 = const.tile([PER, 1], F32, tag="e2")
        nc.vector.tensor_mul(e2, S2_sb, rZ)
        mu2 = const.tile([PER, 1], F32, tag="mu2")
        nc.vector.tensor_mul(mu2, mu, mu)
        var = const.tile([PER, 1], F32, tag="var")
        nc.vector.tensor_scalar(
            out=var, in0=e2, scalar1=mu2, scalar2=1e-7, op0=ALU.subtract, op1=ALU.max
        )

        tv = const.tile([PER, NB], F32, tag="tv")
        nc.vector.tensor_scalar(
            out=tv, in0=sig2_sb, scalar1=var, scalar2=None, op0=ALU.add
        )
        dmu = const.tile([PER, NB], F32, tag="dmu")
        nc.vector.tensor_scalar(
            out=dmu, in0=bmu_sb, scalar1=mu, scalar2=None, op0=ALU.subtract
        )
        dmu2 = const.tile([PER, NB], F32, tag="dmu2")
        nc.vector.tensor_mul(dmu2, dmu, dmu)
        rtv = const.tile([PER, NB], F32, tag="rtv")
        nc.vector.reciprocal(rtv, tv)
        arg = const.tile([PER, NB], F32, tag="arg")
        nc.vector.tensor_mul(arg, dmu2, rtv)
        eterm = const.tile([PER, NB], F32, tag="eterm")
        nc.scalar.activation(out=eterm, in_=arg, func=AF.Exp, scale=-0.5)
        srtv = const.tile([PER, NB], F32, tag="srtv")
        nc.scalar.activation(out=srtv, in_=rtv, func=AF.Sqrt)
        coef = const.tile([PER, NB], F32, tag="coef")
        nc.scalar.mul(coef, srtv, INV_SQRT_2PI)
        r_sb = const.tile([PER, NB], F32, tag="r")
        nc.vector.tensor_mul(r_sb, coef, eterm)

        # rT[j, b] = r[b, j]
        rT_ps = pwork.tile([NB, PER], F32, tag="pwork")
        nc.tensor.matmul(rT_ps, lhsT=r_sb, rhs=I_sb[:PER, :PER], start=True, stop=True)
        rT_sb = const.tile([NB, PER], F32, tag="rT")
        nc.vector.tensor_copy(out=rT_sb, in_=rT_ps)

        for b in range(PER):
            bmT_sb = const.tile([NB, D], F32, tag=f"bmT{b}")
            nc.vector.tensor_copy(out=bmT_sb, in_=bm_ps[b])
            c_ps = pwork.tile([1, D], F32, tag="pwork", name=f"c_ps{b}")
            nc.tensor.matmul(
                c_ps, lhsT=rT_sb[:, b : b + 1], rhs=bmT_sb, start=True, stop=True
            )
            c_sb = const.tile([1, D], F32, tag=f"c{b}")
            nc.vector.tensor_copy(out=c_sb, in_=c_ps)
            nc.sync.dma_start(out=out_t[b : b + 1, :], in_=c_sb)

    nc.finalize()
    return nc


_CACHE = {}


def _get_nc():
    if "nc" not in _CACHE:
        _CACHE["nc"] = _build_bass()
    return _CACHE["nc"]


def make_in_maps(query, keys, values, W_enc, G, basis_mu, basis_sigma):
    query = np.ascontiguousarray(np.asarray(query, dtype=np.float32))
    keys = np.ascontiguousarray(np.asarray(keys, dtype=np.float32))
    values = np.ascontiguousarray(np.asarray(values, dtype=np.float32))
    W_enc = np.ascontiguousarray(np.asarray(W_enc, dtype=np.float32))
    G = np.ascontiguousarray(np.asarray(G, dtype=np.float32))
    basis_mu = np.asarray(basis_mu, dtype=np.float32).reshape(1, NB)
    basis_sigma = np.asarray(basis_sigma, dtype=np.float32).reshape(1, NB)

    ident = np.eye(128, dtype=np.float32)
    pshift = 1.0 / (2.0 * L)
    pos = np.linspace(pshift, 1.0 - pshift, L).astype(np.float32)
    post = np.zeros((128, NT), dtype=np.float32)
    for t in range(NT):
        n = min(128, L - t * 128)
        post[:n, t] = pos[t * 128 : t * 128 + n]
    bmu4 = np.ascontiguousarray(np.tile(basis_mu, (PER, 1)))
    bsig4 = np.ascontiguousarray(np.tile(basis_sigma, (PER, 1)))

    in_maps = []
    for c in range(NCORES):
        sl = slice(c * PER, (c + 1) * PER)
        in_maps.append(
            {
                "keys": np.ascontiguousarray(keys[sl]),
                "values": np.ascontiguousarray(values[sl]),
                "q": np.ascontiguousarray(query[sl, 0, :]),
                "W": W_enc,
                "G": G,
                "bmu": bmu4,
                "bsig": bsig4,
                "ident": ident,
                "post": post,
            }
        )
    return in_maps


def kernel(query, keys, values, mask, W_enc, G, basis_mu, basis_sigma, **_kw):
    nc = _get_nc()
    in_maps = make_in_maps(query, keys, values, W_enc, G, basis_mu, basis_sigma)
    res = run_bass_kernel_spmd(nc, in_maps, core_ids=list(range(NCORES))).results
    out = np.stack([np.asarray(res[c]["out"]) for c in range(NCORES)])  # (8, PER, D)
    return out.reshape(B, 1, D).astype(np.float32)


# revision 19
# speedup vs baseline: 1.0214x; 1.0214x over previous
"""Trainium2 Bass kernel for nn_ContinuousAttention (B=32, L=2999, D=512, NB=16).

Math (per example b):
    u      = W_enc @ q[b]                      (D,)
    s[l]   = keys[b,l,:] . u / sqrt(D)         (L,)   raw scores
    w[l]   = exp(s[l])                          -- no max-subtraction needed:
                                                  s ~ N(0,1), |s| < ~6, exp safe
    Z      = sum w;  S1 = sum w*pos;  S2 = sum w*pos^2
    mu     = S1/Z;  var = clip(S2/Z - mu^2, 1e-7)
    tv_j   = var + basis_sigma_j^2
    r_j    = (1/sqrt(2pi)) / sqrt(tv_j) * exp(-0.5 (mu - mu_j)^2 / tv_j)
    BmatT  = G^T @ values[b]                   (NB, D)  [= (values^T G)^T]
    c[b]   = r . BmatT                         (D,)

Sharding: data-parallel over batch, 4 examples per core x 8 cores.
The only large traffic is keys+values (24.6 MB each per core), streamed
once from HBM in 1 MiB chunks at the per-core HBM roofline (~358 GB/s,
so ~140 us/core is the floor). Score dot-products are spread over three
engines (GpSimd mul + DVE reduce on even sub-tiles, DVE mul + ACT
accumulate-reduce on odd ones); Bmat runs on the PE with G stationary;
the softmax-statistics partition reduction is a ones-vector PE matmul
done per example so it overlaps the remaining stream. Only a short
scalar chain (~20 tiny ops) plus the final combines remain after the
last DMA.
"""

import numpy as np
from contextlib import ExitStack

import concourse.bass as bass
import concourse.bacc as bacc
import concourse.tile as tile
from concourse import mybir
from concourse.bass_utils import run_bass_kernel_spmd

F32 = mybir.dt.float32
AF = mybir.ActivationFunctionType
ALU = mybir.AluOpType

B, L, D, NB = 32, 2999, 512, 16
NCORES = 8
PER = B // NCORES              # 4 examples per core
NT = (L + 127) // 128          # 24 row-tiles of 128
LAST = L - (NT - 1) * 128      # 55 rows in the last tile
INV_SQRT_D = float(1.0 / np.sqrt(float(D)))
INV_SQRT_2PI = float(1.0 / np.sqrt(2.0 * np.pi))
NEG_BIG = -1.0e4               # pad score; exp(NEG_BIG/sqrt(D)) == 0 in f32

# 1 MiB DMA chunks: (row0, n_full_128_subtiles, tail_rows)
CHUNKS = []
_r = 0
while _r < L:
    _full = min(4, (L - _r) // 128)
    _tail = 0 if _full == 4 else L - _r - _full * 128
    CHUNKS.append((_r, _full, _tail))
    _r += _full * 128 + _tail


def _build_bass():
    # Bacc (not raw Bass): its compile pipeline splits multi-wait sync infos
    # into event semaphores, which the TRN2 BIR verifier requires for the
    # Tile kernel-tail drain.
    nc = bacc.Bacc(None, target_bir_lowering=False)
    keys_t = nc.declare_dram_parameter("keys", [PER, L, D], F32, isOutput=False)
    vals_t = nc.declare_dram_parameter("values", [PER, L, D], F32, isOutput=False)
    q_t = nc.declare_dram_parameter("q", [PER, D], F32, isOutput=False)
    W_t = nc.declare_dram_parameter("W", [D, D], F32, isOutput=False)
    G_t = nc.declare_dram_parameter("G", [L, NB], F32, isOutput=False)
    bmu_t = nc.declare_dram_parameter("bmu", [PER, NB], F32, isOutput=False)
    bsig_t = nc.declare_dram_parameter("bsig", [PER, NB], F32, isOutput=False)
    id_t = nc.declare_dram_parameter("ident", [128, 128], F32, isOutput=False)
    pos_t = nc.declare_dram_parameter("post", [128, NT], F32, isOutput=False)
    out_t = nc.declare_dram_parameter("out", [PER, D], F32, isOutput=True)

    with ExitStack() as ctx:
        tc = ctx.enter_context(tile.TileContext(nc))
        const = ctx.enter_context(tc.tile_pool(name="const", bufs=1))
        kpool = ctx.enter_context(tc.tile_pool(name="kpool", bufs=8))
        vpool = ctx.enter_context(tc.tile_pool(name="vpool", bufs=8))
        spool = ctx.enter_context(tc.tile_pool(name="spool", bufs=4))
        pwork = ctx.enter_context(tc.tile_pool(name="pwork", bufs=3, space="PSUM"))
        pbm = ctx.enter_context(tc.tile_pool(name="pbm", bufs=4, space="PSUM"))

        # ------- constants (ACT HWDGE ring; the sync ring is for streams) ----
        I_sb = const.tile([128, 128], F32, tag="I")
        nc.scalar.dma_start(out=I_sb, in_=id_t[:, :])
        pos_sb = const.tile([128, NT], F32, tag="pos")
        nc.scalar.dma_start(out=pos_sb, in_=pos_t[:, :])
        ones_sb = const.tile([128, 1], F32, tag="ones")
        nc.vector.memset(ones_sb, 1.0)
        bmu_sb = const.tile([PER, NB], F32, tag="bmu")
        nc.scalar.dma_start(out=bmu_sb, in_=bmu_t[:, :])
        bsig_sb = const.tile([PER, NB], F32, tag="bsig")
        nc.scalar.dma_start(out=bsig_sb, in_=bsig_t[:, :])
        sig2_sb = const.tile([PER, NB], F32, tag="sig2")
        nc.vector.tensor_mul(sig2_sb, bsig_sb, bsig_sb)

        # G (L, NB) -> (128, NT, NB); last tile has LAST valid rows
        G_sb = const.tile([128, NT, NB], F32, tag="G")
        nc.scalar.dma_start(
            out=G_sb[:, 0 : NT - 1, :],
            in_=G_t[0 : (NT - 1) * 128, :].rearrange("(t p) n -> p t n", p=128),
        )
        nc.scalar.dma_start(out=G_sb[:LAST, NT - 1, :], in_=G_t[(NT - 1) * 128 : L, :])

        q_sb = const.tile([PER, D], F32, tag="q")
        nc.scalar.dma_start(out=q_sb, in_=q_t[:, :])

        # W (D, D) row-tiled: w_sb[p, dt, e] = W[dt*128+p, e]
        w_sb = const.tile([128, 4, D], F32, tag="W")
        nc.scalar.dma_start(out=w_sb, in_=W_t[:, :].rearrange("(t p) e -> p t e", p=128))

        # qT[p, et, b] = q[b, et*128+p]
        qT_sb = const.tile([128, 4, PER], F32, tag="qT")
        for et in range(4):
            tp = pwork.tile([128, PER], F32, tag="pwork")
            nc.tensor.transpose(tp, q_sb[:, et * 128 : (et + 1) * 128], I_sb[:PER, :PER])
            nc.vector.tensor_copy(out=qT_sb[:, et, :], in_=tp)

        # wT_sb[p, et, d] = W[d, et*128+p]  (transpose of W, e on partitions)
        wT_sb = const.tile([128, 4, D], F32, tag="WT")
        for dt in range(4):
            for et in range(4):
                tp = pwork.tile([128, 128], F32, tag="pwork")
                nc.tensor.transpose(tp, w_sb[:, dt, et * 128 : (et + 1) * 128], I_sb)
                nc.vector.tensor_copy(
                    out=wT_sb[:, et, dt * 128 : (dt + 1) * 128], in_=tp
                )

        # U[p, dm, b] = u_b[dm*128+p] = sum_e W[dm*128+p, e] q[b, e]
        U_sb = const.tile([128, 4, PER], F32, tag="U")
        for dm in range(4):
            up = pwork.tile([128, PER], F32, tag="pwork")
            for et in range(4):
                nc.tensor.matmul(
                    up,
                    lhsT=wT_sb[:, et, dm * 128 : (dm + 1) * 128],
                    rhs=qT_sb[:, et, :],
                    start=(et == 0),
                    stop=(et == 3),
                )
            nc.vector.tensor_copy(out=U_sb[:, dm, :], in_=up)

        # UT[b, d] = u_b[d]
        UT_sb = const.tile([PER, D], F32, tag="UT")
        for dm in range(4):
            tp = pwork.tile([PER, 128], F32, tag="pwork")
            nc.tensor.transpose(tp, U_sb[:, dm, :], I_sb)
            nc.vector.tensor_copy(out=UT_sb[:, dm * 128 : (dm + 1) * 128], in_=tp)

        # u_sb[p, b, d] = u_b[d] broadcast across all 128 partitions.
        # Step 1: select row b of UT into partition 0 (lhsT = I4 column b).
        # Step 2: outer-product with a ones row to replicate across partitions.
        ones_row = const.tile([1, 128], F32, tag="ones_row")
        nc.vector.memset(ones_row, 1.0)
        u_sb = const.tile([128, PER, D], F32, tag="u")
        for b in range(PER):
            ur_ps = pwork.tile([1, D], F32, tag="pwork", name=f"ur_ps{b}")
            nc.tensor.matmul(
                ur_ps, lhsT=I_sb[:PER, b : b + 1], rhs=UT_sb, start=True, stop=True
            )
            ur_sb = const.tile([1, D], F32, tag="ur", name=f"ur{b}")
            nc.vector.tensor_copy(out=ur_sb, in_=ur_ps)
            ub = pwork.tile([128, D], F32, tag="pwork", name=f"ub{b}")
            nc.tensor.matmul(ub, lhsT=ones_row, rhs=ur_sb, start=True, stop=True)
            nc.vector.tensor_copy(out=u_sb[:, b, :], in_=ub)

        # ---------------- main streams ----------------
        scores_sb = const.tile([128, PER, NT], F32, tag="scores")
        nc.vector.memset(scores_sb, NEG_BIG)
        # wst[p, b, s, t]: s=0 -> w, s=1 -> w*pos, s=2 -> w*pos^2
        # (b-major: each example's 3*NT stat block is contiguous)
        wst_sb = const.tile([128, PER, 3, NT], F32, tag="wst")
        # per-example partition-reduced stats land in st_all[0, :, b]
        st_all = const.tile([1, 3, PER], F32, tag="st_all")
        bm_ps = [
            pbm.tile([NB, D], F32, tag="pbm", name=f"bm_ps{b}") for b in range(PER)
        ]

        for b in range(PER):
            for (r0, nfull, tailr) in CHUNKS:
                k_tile = kpool.tile([128, 4, D], F32, tag="ktile")
                v_tile = vpool.tile([128, 4, D], F32, tag="vtile")
                nc.sync.dma_start(
                    out=k_tile[:, 0:nfull, :],
                    in_=keys_t[b, r0 : r0 + nfull * 128, :].rearrange(
                        "(t p) d -> p t d", p=128
                    ),
                )
                nc.sync.dma_start(
                    out=v_tile[:, 0:nfull, :],
                    in_=vals_t[b, r0 : r0 + nfull * 128, :].rearrange(
                        "(t p) d -> p t d", p=128
                    ),
                )
                if tailr:
                    nc.sync.dma_start(
                        out=k_tile[:tailr, nfull, :],
                        in_=keys_t[b, r0 + nfull * 128 : L, :],
                    )
                    nc.sync.dma_start(
                        out=v_tile[:tailr, nfull, :],
                        in_=vals_t[b, r0 + nfull * 128 : L, :],
                    )
                nsub = nfull + (1 if tailr else 0)
                for s in range(nsub):
                    t = r0 // 128 + s
                    P = 128 if s < nfull else tailr
                    # keys[l,:] . u  -> score per row, spread over three
                    # engines so each stays under the DMA pace: even
                    # sub-tiles GpSimd-mul + DVE-reduce, odd sub-tiles
                    # DVE-mul + ACT accumulate-reduce.
                    scr = spool.tile([128, D], F32, tag="scr")
                    if s % 2 == 0:
                        nc.gpsimd.tensor_mul(
                            scr[:P, :], k_tile[:P, s, :], u_sb[:P, b, :]
                        )
                        nc.vector.tensor_reduce(
                            out=scores_sb[:P, b, t : t + 1],
                            in_=scr[:P, :],
                            axis=mybir.AxisListType.X,
                            op=ALU.add,
                        )
                    else:
                        nc.vector.tensor_mul(
                            scr[:P, :], k_tile[:P, s, :], u_sb[:P, b, :]
                        )
                        nc.scalar.activation(
                            out=scr[:P, :],
                            in_=scr[:P, :],
                            func=AF.Copy,
                            accum_out=scores_sb[:P, b, t : t + 1],
                        )
                    nc.tensor.matmul(
                        bm_ps[b],
                        lhsT=G_sb[:P, t, :],
                        rhs=v_tile[:P, s, :],
                        start=(t == 0),
                        stop=(t == NT - 1),
                    )

            # per-example 128-wide stats work, overlapping later streams
            nc.scalar.activation(
                out=wst_sb[:, b, 0, :],
                in_=scores_sb[:, b, :],
                func=AF.Exp,
                scale=INV_SQRT_D,
            )
            nc.vector.tensor_mul(wst_sb[:, b, 1, :], wst_sb[:, b, 0, :], pos_sb)
            nc.vector.tensor_mul(wst_sb[:, b, 2, :], wst_sb[:, b, 1, :], pos_sb)
            st_ps = pwork.tile([1, 3, NT], F32, tag="pwork", name=f"st_ps{b}")
            nc.tensor.matmul(
                st_ps, lhsT=ones_sb, rhs=wst_sb[:, b, :, :], start=True, stop=True
            )
            nc.vector.tensor_reduce(
                out=st_all[:, :, b], in_=st_ps, axis=mybir.AxisListType.X, op=ALU.add
            )

        # ---------------- tail (short scalar chain) ----------------
        # (1, PER) stat rows -> (PER, 1) columns via tiny PE transposes
        zs = []
        for s in range(3):
            tp = pwork.tile([PER, 1], F32, tag="pwork", name=f"zt{s}")
            nc.tensor.matmul(
                tp, lhsT=st_all[:, s, :], rhs=I_sb[:1, :1], start=True, stop=True
            )
            z_sb = const.tile([PER, 1], F32, tag=f"zs{s}")
            nc.vector.tensor_copy(out=z_sb, in_=tp)
            zs.append(z_sb)
        Z_sb, S1_sb, S2_sb = zs

        rZ = const.tile([PER, 1], F32, tag="rZ")
        nc.vector.reciprocal(rZ, Z_sb)
        mu = const.tile([PER, 1], F32, tag="mu")
        nc.vector.tensor_mul(mu, S1_sb, rZ)
        e2 = const.tile([PER, 1], F32, tag="e2")
        nc.vector.tensor_mul(e2, S2_sb, rZ)
        mu2 = const.tile([PER, 1], F32, tag="mu2")
        nc.vector.tensor_mul(mu2, mu, mu)
        var = const.tile([PER, 1], F32, tag="var")
        nc.vector.tensor_sub(var, e2, mu2)
        nc.vector.tensor_scalar_max(var, var, 1e-7)

        tv = const.tile([PER, NB], F32, tag="tv")
        nc.vector.tensor_scalar(
            out=tv, in0=sig2_sb, scalar1=var, scalar2=None, op0=ALU.add
        )
        dmu = const.tile([PER, NB], F32, tag="dmu")
        nc.vector.tensor_scalar(
            out=dmu, in0=bmu_sb, scalar1=mu, scalar2=None, op0=ALU.subtract
        )
        dmu2 = const.tile([PER, NB], F32, tag="dmu2")
        nc.vector.tensor_mul(dmu2, dmu, dmu)
        rtv = const.tile([PER, NB], F32, tag="rtv")
        nc.vector.reciprocal(rtv, tv)
        arg = const.tile([PER, NB], F32, tag="arg")
        nc.vector.tensor_mul(arg, dmu2, rtv)
        eterm = const.tile([PER, NB], F32, tag="eterm")
        nc.scalar.activation(out=eterm, in_=arg, func=AF.Exp, scale=-0.5)
        srtv = const.tile([PER, NB], F32, tag="srtv")
        nc.scalar.activation(out=srtv, in_=rtv, func=AF.Sqrt)
        coef = const.tile([PER, NB], F32, tag="coef")
        nc.scalar.mul(coef, srtv, INV_SQRT_2PI)
        r_sb = const.tile([PER, NB], F32, tag="r")
        nc.vector.tensor_mul(r_sb, coef, eterm)

        # rT[j, b] = r[b, j]
        rT_ps = pwork.tile([NB, PER], F32, tag="pwork")
        nc.tensor.matmul(rT_ps, lhsT=r_sb, rhs=I_sb[:PER, :PER], start=True, stop=True)
        rT_sb = const.tile([NB, PER], F32, tag="rT")
        nc.vector.tensor_copy(out=rT_sb, in_=rT_ps)

        for b in range(PER):
            bmT_sb = const.tile([NB, D], F32, tag=f"bmT{b}")
            nc.vector.tensor_copy(out=bmT_sb, in_=bm_ps[b])
            c_ps = pwork.tile([1, D], F32, tag="pwork", name=f"c_ps{b}")
            nc.tensor.matmul(
                c_ps, lhsT=rT_sb[:, b : b + 1], rhs=bmT_sb, start=True, stop=True
            )
            c_sb = const.tile([1, D], F32, tag=f"c{b}")
            nc.vector.tensor_copy(out=c_sb, in_=c_ps)
            nc.sync.dma_start(out=out_t[b : b + 1, :], in_=c_sb)

    nc.finalize()
    return nc


_CACHE = {}


def _get_nc():
    if "nc" not in _CACHE:
        _CACHE["nc"] = _build_bass()
    return _CACHE["nc"]


def make_in_maps(query, keys, values, W_enc, G, basis_mu, basis_sigma):
    query = np.ascontiguousarray(np.asarray(query, dtype=np.float32))
    keys = np.ascontiguousarray(np.asarray(keys, dtype=np.float32))
    values = np.ascontiguousarray(np.asarray(values, dtype=np.float32))
    W_enc = np.ascontiguousarray(np.asarray(W_enc, dtype=np.float32))
    G = np.ascontiguousarray(np.asarray(G, dtype=np.float32))
    basis_mu = np.asarray(basis_mu, dtype=np.float32).reshape(1, NB)
    basis_sigma = np.asarray(basis_sigma, dtype=np.float32).reshape(1, NB)

    ident = np.eye(128, dtype=np.float32)
    pshift = 1.0 / (2.0 * L)
    pos = np.linspace(pshift, 1.0 - pshift, L).astype(np.float32)
    post = np.zeros((128, NT), dtype=np.float32)
    for t in range(NT):
        n = min(128, L - t * 128)
        post[:n, t] = pos[t * 128 : t * 128 + n]
    bmu4 = np.ascontiguousarray(np.tile(basis_mu, (PER, 1)))
    bsig4 = np.ascontiguousarray(np.tile(basis_sigma, (PER, 1)))

    in_maps = []
    for c in range(NCORES):
        sl = slice(c * PER, (c + 1) * PER)
        in_maps.append(
            {
                "keys": np.ascontiguousarray(keys[sl]),
                "values": np.ascontiguousarray(values[sl]),
                "q": np.ascontiguousarray(query[sl, 0, :]),
                "W": W_enc,
                "G": G,
                "bmu": bmu4,
                "bsig": bsig4,
                "ident": ident,
                "post": post,
            }
        )
    return in_maps


def kernel(query, keys, values, mask, W_enc, G, basis_mu, basis_sigma, **_kw):
    nc = _get_nc()
    in_maps = make_in_maps(query, keys, values, W_enc, G, basis_mu, basis_sigma)
    res = run_bass_kernel_spmd(nc, in_maps, core_ids=list(range(NCORES))).results
    out = np.stack([np.asarray(res[c]["out"]) for c in range(NCORES)])  # (8, PER, D)
    return out.reshape(B, 1, D).astype(np.float32)
